# revision 20
# baseline (speedup 1.0000x reference)
"""Multi-head self-attention (RoPE, causal) Trainium2 kernel, 8-way sharded.

Sharding: data-parallel over batch (B=2) x tensor-parallel over head groups
(16 heads -> 4 groups of 4). Core c handles batch c//4, heads 4*(c%4)..+4.
Each core computes q/k/v projections for its heads, RoPE, causal-softmax
attention, and a Megatron-style row-parallel partial of the output
projection; the host sums the 4 partials per batch.

On-device layout trick: scores are computed transposed (scores^T[k, q]) so
that softmax-denominator accumulation and the attn@V contraction both run
as natural matmuls with zero on-chip transposes. x is fed pre-transposed
[C, T] from the host; rotate_half is an SBUF->SBUF partition-gather DMA
with the sign folded into the sin table. Attention is processed in two
query halves so the first half starts while input DMA/projections for the
second half are still in flight, and the first half of the output
projection overlaps the second attention half.
"""
import sys
for _p in ("/opt/trn_rl_repo",):
    if _p not in sys.path:
        sys.path.insert(0, _p)

import numpy as np
from contextlib import ExitStack

import concourse.bacc as bacc
import concourse.mybir as mybir
import concourse.tile as tile
from concourse.bass_utils import run_bass_kernel_spmd

F32 = mybir.dt.float32
F32R = mybir.dt.float32r
AF = mybir.ActivationFunctionType

B, T, C = 2, 2048, 1024
H, Dh = 16, 64
HL = 4                      # heads per core
CK = C // 128               # 8 contraction k-tiles for projections
TTL = T // 128              # 16 T-tiles / kv k-tiles
HT = T // 2                 # 1024, the attention q-half width
N_CORES = 8


def build_nc():
    nc = bacc.Bacc("TRN2", target_bir_lowering=False, debug=False, num_devices=N_CORES)

    xt = nc.declare_dram_parameter("xt", [C, T], F32R, isOutput=False)
    wqkv = nc.declare_dram_parameter("wqkv", [C, 4 * 128 + HL * Dh], F32R, isOutput=False)
    wo = nc.declare_dram_parameter("wo", [HL * Dh, C], F32R, isOutput=False)
    cosT = nc.declare_dram_parameter("cosT", [128, T], F32R, isOutput=False)
    sinT = nc.declare_dram_parameter("sinT", [128, T], F32R, isOutput=False)
    maskT = nc.declare_dram_parameter("maskT", [128, 128], F32R, isOutput=False)
    ones64 = nc.declare_dram_parameter("ones64", [1, Dh], F32R, isOutput=False)
    ones4 = nc.declare_dram_parameter("ones4", [128, TTL * HL], F32R, isOutput=False)
    rotT = nc.declare_dram_parameter("rotT", [128, 128], F32R, isOutput=False)
    out = nc.declare_dram_parameter("out", [T, C], F32, isOutput=True)

    with nc.allow_low_precision("fp32r matmul pipeline"), \
         tile.TileContext(nc) as tc, ExitStack() as octx:
        consts = octx.enter_context(tc.tile_pool(name="consts", bufs=1))
        v_pool = octx.enter_context(tc.tile_pool(name="v", bufs=1))
        qkt_pool = octx.enter_context(tc.tile_pool(name="qkt", bufs=1))
        ao_pool = octx.enter_context(tc.tile_pool(name="ao", bufs=1))
        p_pool = octx.enter_context(tc.tile_pool(name="pb", bufs=4))
        avsb_pool = octx.enter_context(tc.tile_pool(name="avsbp", bufs=2))
        rec_pool = octx.enter_context(tc.tile_pool(name="recp", bufs=2))
        sc_ps = octx.enter_context(tc.tile_pool(name="scps", bufs=2, space="PSUM"))
        av_ps = octx.enter_context(tc.tile_pool(name="avps", bufs=1, space="PSUM"))

        mask_t = consts.tile([128, 128], F32R, tag="mask")
        rotT_t = consts.tile([128, 128], F32R, tag="rotT")
        ones64_t = consts.tile([1, Dh], F32R, tag="ones64")

        vext_t = v_pool.tile([128, TTL, HL, Dh + 1], F32R, tag="vext", name="vext")
        vext = [vext_t[:, t_] for t_ in range(TTL)]
        # qkt[mt][half]: mt 0=Q heads01, 1=K heads01, 2=Q heads23, 3=K heads23
        qkt = [[qkt_pool.tile([128, HT], F32R, tag=f"qkt{m}_{hf}", name=f"qkt{m}_{hf}")
                for hf in range(2)] for m in range(4)]
        ao = [ao_pool.tile([128, T], F32R, tag=f"ao{i}", name=f"ao{i}") for i in range(2)]

        state = {"pending": None}

        def emit_normalize(h_, half_, avsb_):
            pr_ = 64 * (h_ % 2)
            rec = rec_pool.tile([1, HT], F32R, tag="rec", name="rec")
            nc.vector.reciprocal(rec[:], avsb_[Dh:Dh + 1, :])
            bc = sc_ps.tile([128, HT], F32, tag="sc", name="bc")
            for j in range(2):
                nc.tensor.matmul(bc[0:Dh, 512 * j:512 * (j + 1)],
                                 ones64_t[:], rec[:, 512 * j:512 * (j + 1)],
                                 start=True, stop=True)
            nc.vector.tensor_mul(ao[h_ // 2][pr_:pr_ + 64, HT * half_:HT * (half_ + 1)],
                                 avsb_[0:Dh, :], bc[0:Dh, :].bitcast(F32R))

        def attn_unit(h, half, fillers=()):
            """scores^T/exp/mask/attn@V for head h, query half `half`.
            `fillers` are independent emission closures injected one-per-strip
            to keep PE fed while the softmax pipeline ramps."""
            fillers = list(fillers)
            qrmt, krmt = (0, 1) if h < 2 else (2, 3)
            pr = 64 * (h % 2)
            q_lo = HT * half
            av = av_ps.tile([128, HT], F32, tag="av", name="av")
            n_strips = 8 if half == 0 else 16

            def emit_av(m, p_, cs_):
                seg = cs_ - q_lo   # local column within av tile
                while seg < HT:
                    seg_end = min((seg // 512 + 1) * 512, HT)
                    jb = 2 * half + seg // 512
                    nc.tensor.matmul(
                        av[0:Dh + 1, seg:seg_end],
                        vext[m][:, h, :],
                        p_[:, seg - (cs_ - q_lo):seg_end - (cs_ - q_lo)],
                        start=(m == 0), stop=(m == 4 * jb + 3))
                    seg = seg_end

            prev = None
            for m in range(n_strips):
                cs = max(q_lo, 128 * m)
                W = q_lo + HT - cs
                kr_t = qkt[krmt][m // 8]
                kc = 128 * m - HT * (m // 8)
                sc = sc_ps.tile([128, HT], F32, tag="sc", name="sc")
                j = 0
                while 512 * j < W:
                    n = min(512, W - 512 * j)
                    qc = cs - q_lo + 512 * j
                    nc.tensor.matmul(
                        sc[:, 512 * j:512 * j + n],
                        kr_t[pr:pr + 64, kc:kc + 128],
                        qkt[qrmt][half][pr:pr + 64, qc:qc + n],
                        start=True, stop=True)
                    j += 1
                p = p_pool.tile([128, HT], F32R, tag="p", name="p")
                nc.scalar.activation(p[:, 0:W], sc[:, 0:W], AF.Exp, scale=0.125)
                if cs == 128 * m:
                    nc.vector.tensor_mul(p[:, 0:128], p[:, 0:128], mask_t[:])
                if m == 0 and state["pending"] is not None:
                    emit_normalize(*state["pending"])
                    state["pending"] = None
                if prev is not None:
                    emit_av(*prev)
                prev = (m, p, cs)
                if m >= 1 and fillers:
                    fillers.pop(0)()
            emit_av(*prev)
            for f in fillers:
                f()
            avsb = avsb_pool.tile([Dh + 1, HT], F32R, tag="avsb", name="avsb")
            nc.vector.tensor_copy(avsb[0:Dh + 1, :], av[0:Dh + 1, :])
            state["pending"] = (h, half, avsb)

        with tc.tile_pool(name="xtp", bufs=1) as xt_pool, \
             tc.tile_pool(name="wqkp", bufs=1) as wqk_pool, \
             tc.tile_pool(name="ropetab", bufs=1) as rtab_pool, \
             tc.tile_pool(name="ropetmp", bufs=2) as rtmp_pool, \
             tc.tile_pool(name="projps", bufs=2, space="PSUM") as proj_ps:

            wqkv_t = [wqk_pool.tile([128, 512 + HL * Dh], F32R, tag=f"wqkv{k}", name=f"wqkv{k}")
                      for k in range(CK)]
            wqk_t = [w[:, 0:512] for w in wqkv_t]
            wv_t = [w[:, 512:512 + HL * Dh] for w in wqkv_t]

            xt_t = [xt_pool.tile([128, T], F32R, tag=f"xt{k}", name=f"xt{k}")
                    for k in range(CK)]

            def xt_dma(hf):
                for k in range(CK):
                    nc.sync.dma_start(xt_t[k][:, HT * hf:HT * (hf + 1)],
                                      xt[128 * k:128 * (k + 1), HT * hf:HT * (hf + 1)])

            # DMA emission order tracks the attention-A critical path
            cos_t = rtab_pool.tile([128, T], F32R, tag="cos")
            sin_t = rtab_pool.tile([128, T], F32R, tag="sin")
            for k in range(CK):
                nc.sync.dma_start(wqkv_t[k][:], wqkv[128 * k:128 * (k + 1), :])
                nc.sync.dma_start(xt_t[k][:, 0:HT], xt[128 * k:128 * (k + 1), 0:HT])
            nc.sync.dma_start(cos_t[:], cosT[:])
            nc.sync.dma_start(sin_t[:], sinT[:])
            nc.sync.dma_start(mask_t[:], maskT[:])
            nc.sync.dma_start(rotT_t[:], rotT[:])
            nc.sync.dma_start(ones64_t[:], ones64[:])

            rope_pending = []

            def emit_rope(m, n):
                """rotate-half via a PE permutation matmul, then the cos/sin
                elementwise combine. Emitted one projection group late so the
                PSUM->SBUF copy has drained."""
                dst = qkt[m][n // 2]
                src = dst[:, 512 * (n % 2):512 * (n % 2 + 1)]
                rps = sc_ps.tile([128, 512], F32, tag="sc", name="rps")
                nc.tensor.matmul(rps[:], rotT_t[:], src, start=True, stop=True)
                rot = rtmp_pool.tile([128, 512], F32R, tag="rot", name="rot")
                nc.vector.tensor_mul(rot[:], rps[:].bitcast(F32R),
                                     sin_t[:, 512 * n:512 * (n + 1)])
                nc.gpsimd.tensor_mul(src, src, cos_t[:, 512 * n:512 * (n + 1)])
                nc.gpsimd.tensor_add(src, src, rot[:])

            def flush_rope():
                while rope_pending:
                    emit_rope(*rope_pending.pop(0))

            def proj_group(m, n):
                pp = proj_ps.tile([128, 512], F32, tag="pp", name="pp")
                for k in range(CK):
                    nc.tensor.matmul(pp[:], wqk_t[k][:, 128 * m:128 * (m + 1)],
                                     xt_t[k][:, 512 * n:512 * (n + 1)],
                                     start=(k == 0), stop=(k == CK - 1))
                dst = qkt[m][n // 2]
                nc.vector.tensor_copy(dst[:, 512 * (n % 2):512 * (n % 2 + 1)], pp[:])
                pending = rope_pending[:]
                rope_pending.clear()
                rope_pending.append((m, n))
                for pmn in pending:
                    emit_rope(*pmn)

            def vproj_tile(t_):
                flush_rope()
                vp = proj_ps.tile([128, HL * Dh], F32, tag="pp", name="vp")
                for k in range(CK):
                    nc.tensor.matmul(vp[:], xt_t[k][:, 128 * t_:128 * (t_ + 1)], wv_t[k][:],
                                     start=(k == 0), stop=(k == CK - 1))
                nc.vector.tensor_copy(
                    vext[t_][:, :, 0:Dh],
                    vp[:].rearrange("p (h d) -> p h d", h=HL))
                nc.sync.dma_start(
                    vext[t_][:, :, Dh:Dh + 1],
                    ones4[:, HL * t_:HL * (t_ + 1)].rearrange("p (h x) -> p h x", x=1))

            # heads01 projections + V for the first query half, then attention-A
            # units with the remaining projection work injected between strips
            # (PE executes in emission order, so attention must be emitted as
            # soon as its dependencies are, with later work woven in as filler)
            def pg(m, n):
                return lambda: proj_group(m, n)

            def vt(t_):
                return lambda: vproj_tile(t_)

            proj_group(0, 0)
            proj_group(1, 0)
            proj_group(0, 1)
            proj_group(1, 1)
            for t_ in range(0, 4):
                vproj_tile(t_)
            attn_unit(0, 0, [vt(4), vt(5), vt(6), vt(7), pg(2, 0), pg(2, 1)])
            xt_dma(1)
            attn_unit(1, 0, [pg(3, 0), pg(3, 1), pg(0, 2), pg(1, 2), vt(8), vt(9)])
            attn_unit(2, 0, [vt(10), vt(11), pg(0, 3), pg(1, 3), vt(12), vt(13)])
            attn_unit(3, 0, [vt(14), vt(15), pg(2, 2), pg(2, 3), pg(3, 2), pg(3, 3)])
            flush_rope()

        with tc.tile_pool(name="wop", bufs=1) as wo_pool, \
             tc.tile_pool(name="outsb", bufs=3) as out_pool, \
             tc.tile_pool(name="opps", bufs=2, space="PSUM") as op_ps:
            wo_t = [wo_pool.tile([128, C], F32R, tag=f"wo{i}", name=f"wo{i}")
                    for i in range(2)]
            for i in range(2):
                nc.sync.dma_start(wo_t[i][:], wo[128 * i:128 * (i + 1), :])

            def outproj_tile(t_, engine, alt=False):
                osb = out_pool.tile([128, C], F32, tag="osb", name="osb")
                for n in range(2):
                    if alt:
                        op = sc_ps.tile([128, 512], F32, tag="sc", name="op")
                    else:
                        op = op_ps.tile([128, 512], F32, tag="op", name="op")
                    nc.tensor.matmul(op[:],
                                     ao[0][:, 128 * t_:128 * (t_ + 1)],
                                     wo_t[0][:, 512 * n:512 * (n + 1)],
                                     start=True, stop=False)
                    nc.tensor.matmul(op[:],
                                     ao[1][:, 128 * t_:128 * (t_ + 1)],
                                     wo_t[1][:, 512 * n:512 * (n + 1)],
                                     start=False, stop=True)
                    if engine == "v":
                        nc.vector.tensor_copy(osb[:, 512 * n:512 * (n + 1)], op[:])
                    else:
                        nc.scalar.copy(osb[:, 512 * n:512 * (n + 1)], op[:])
                nc.sync.dma_start(out[128 * t_:128 * (t_ + 1), :], osb[:])

            def ot(t_, eng):
                return lambda: outproj_tile(t_, eng)

            attn_unit(0, 1)
            attn_unit(1, 1, [ot(0, "v"), ot(1, "v"), ot(2, "v")])
            attn_unit(2, 1, [ot(3, "v"), ot(4, "v"), ot(5, "v")])
            attn_unit(3, 1, [ot(6, "v"), ot(7, "v")])
            emit_normalize(*state["pending"])
            state["pending"] = None
            for t_ in range(8, TTL):
                outproj_tile(t_, "s" if t_ % 2 == 0 else "v", alt=(t_ % 2 == 1))

    nc.finalize()
    return nc


_NC = None


def _get_nc():
    global _NC
    if _NC is None:
        _NC = build_nc()
    return _NC


def _host_tables():
    inv_freq = 1.0 / (10000.0 ** (np.arange(0, Dh, 2, dtype=np.float32) / Dh))  # [32]
    t = np.arange(T, dtype=np.float32)
    freqs = t[:, None] * inv_freq[None, :]                  # [T, 32]
    emb = np.concatenate([freqs, freqs], axis=-1)           # [T, 64]
    cos = np.cos(emb).T.astype(np.float32)                  # [64, T]
    sin = np.sin(emb).T.astype(np.float32)                  # [64, T]
    sin_signed = sin.copy()
    sin_signed[0:32, :] *= -1.0                             # rotate_half sign fold
    cosT = np.concatenate([cos, cos], axis=0)               # [128, T] two head-halves
    sinT = np.concatenate([sin_signed, sin_signed], axis=0)
    maskT = np.triu(np.ones((128, 128), np.float32))        # keep where k <= q
    sigma = np.empty(64, np.int64)
    sigma[0:32] = 2 * np.arange(32) + 1
    sigma[32:64] = 2 * np.arange(32)
    R = np.zeros((128, 128), np.float32)
    for hh in range(2):
        for d in range(64):
            R[64 * hh + d, 64 * hh + sigma[d]] = 1.0
    rotT = np.ascontiguousarray(R.T)
    return cosT, sinT, maskT, rotT


def kernel(x, w_qkv, w_out):
    x = np.asarray(x, dtype=np.float32)
    w_qkv = np.asarray(w_qkv, dtype=np.float32)
    w_out = np.asarray(w_out, dtype=np.float32)
    nc = _get_nc()
    cosT, sinT, maskT, rotT = _host_tables()
    ones64 = np.ones((1, Dh), np.float32)
    ones4 = np.ones((128, TTL * HL), np.float32)

    in_maps = []
    for core in range(N_CORES):
        b = core // 4
        g = core % 4
        heads = [4 * g + l for l in range(HL)]
        qcols = [w_qkv[:, 64 * h:64 * (h + 1)] for h in heads]
        kcols = [w_qkv[:, C + 64 * h:C + 64 * (h + 1)] for h in heads]
        vcols = [w_qkv[:, 2 * C + 64 * h:2 * C + 64 * (h + 1)] for h in heads]
        # m-tiles: Q01 | K01 | Q23 | K23
        wqkv_loc = np.concatenate(
            [qcols[0], qcols[1], kcols[0], kcols[1], qcols[2], qcols[3], kcols[2], kcols[3]]
            + vcols, axis=1).copy()                          # [C, 768]
        wo_loc = np.concatenate([w_out[64 * h:64 * (h + 1), :] for h in heads], axis=0).copy()
        in_maps.append({
            "xt": np.ascontiguousarray(x[b].T),              # [C, T]
            "wqkv": wqkv_loc,
            "wo": wo_loc,
            "cosT": cosT, "sinT": sinT, "maskT": maskT, "rotT": rotT,
            "ones64": ones64, "ones4": ones4,
        })

    res = run_bass_kernel_spmd(nc, in_maps, core_ids=list(range(N_CORES)))
    out_arr = np.zeros((B, T, C), np.float32)
    for core in range(N_CORES):
        out_arr[core // 4] += res.results[core]["out"]
    return out_arr


# revision 21
# speedup vs baseline: 1.0021x; 1.0021x over previous
"""Multi-head self-attention (RoPE, causal) Trainium2 kernel, 8-way sharded.

Sharding: data-parallel over batch (B=2) x tensor-parallel over head groups
(16 heads -> 4 groups of 4). Core c handles batch c//4, heads 4*(c%4)..+4.
Each core computes q/k/v projections for its heads, RoPE, causal-softmax
attention, and a Megatron-style row-parallel partial of the output
projection; the host sums the 4 partials per batch.

On-device layout trick: scores are computed transposed (scores^T[k, q]) so
that softmax-denominator accumulation and the attn@V contraction both run
as natural matmuls with zero on-chip transposes. x is fed pre-transposed
[C, T] from the host; rotate_half is an SBUF->SBUF partition-gather DMA
with the sign folded into the sin table. Attention is processed in two
query halves so the first half starts while input DMA/projections for the
second half are still in flight, and the first half of the output
projection overlaps the second attention half.
"""
import sys
for _p in ("/opt/trn_rl_repo",):
    if _p not in sys.path:
        sys.path.insert(0, _p)

import numpy as np
from contextlib import ExitStack

import concourse.bacc as bacc
import concourse.mybir as mybir
import concourse.tile as tile
from concourse.bass_utils import run_bass_kernel_spmd

F32 = mybir.dt.float32
F32R = mybir.dt.float32r
AF = mybir.ActivationFunctionType

B, T, C = 2, 2048, 1024
H, Dh = 16, 64
HL = 4                      # heads per core
CK = C // 128               # 8 contraction k-tiles for projections
TTL = T // 128              # 16 T-tiles / kv k-tiles
HT = T // 2                 # 1024, the attention q-half width
N_CORES = 8


def build_nc():
    nc = bacc.Bacc("TRN2", target_bir_lowering=False, debug=False, num_devices=N_CORES)

    xt = nc.declare_dram_parameter("xt", [C, T], F32R, isOutput=False)
    wqkv = nc.declare_dram_parameter("wqkv", [C, 4 * 128 + HL * Dh], F32R, isOutput=False)
    wo = nc.declare_dram_parameter("wo", [HL * Dh, C], F32R, isOutput=False)
    cosT = nc.declare_dram_parameter("cosT", [128, T], F32R, isOutput=False)
    sinT = nc.declare_dram_parameter("sinT", [128, T], F32R, isOutput=False)
    maskT = nc.declare_dram_parameter("maskT", [128, 128], F32R, isOutput=False)
    ones64 = nc.declare_dram_parameter("ones64", [1, Dh], F32R, isOutput=False)
    ones4 = nc.declare_dram_parameter("ones4", [128, TTL * HL], F32R, isOutput=False)
    rotT = nc.declare_dram_parameter("rotT", [128, 128], F32R, isOutput=False)
    out = nc.declare_dram_parameter("out", [T, C], F32, isOutput=True)

    with nc.allow_low_precision("fp32r matmul pipeline"), \
         tile.TileContext(nc) as tc, ExitStack() as octx:
        consts = octx.enter_context(tc.tile_pool(name="consts", bufs=1))
        v_pool = octx.enter_context(tc.tile_pool(name="v", bufs=1))
        qkt_pool = octx.enter_context(tc.tile_pool(name="qkt", bufs=1))
        ao_pool = octx.enter_context(tc.tile_pool(name="ao", bufs=1))
        p_pool = octx.enter_context(tc.tile_pool(name="pb", bufs=3))
        avsb_pool = octx.enter_context(tc.tile_pool(name="avsbp", bufs=2))
        rec_pool = octx.enter_context(tc.tile_pool(name="recp", bufs=2))
        sc_ps = octx.enter_context(tc.tile_pool(name="scps", bufs=2, space="PSUM"))
        av_ps = octx.enter_context(tc.tile_pool(name="avps", bufs=1, space="PSUM"))

        mask_t = consts.tile([128, 128], F32R, tag="mask")
        rotT_t = consts.tile([128, 128], F32R, tag="rotT")
        ones64_t = consts.tile([1, Dh], F32R, tag="ones64")

        vext_t = v_pool.tile([128, TTL, HL, Dh + 1], F32R, tag="vext", name="vext")
        vext = [vext_t[:, t_] for t_ in range(TTL)]
        # qkt[mt][half]: mt 0=Q heads01, 1=K heads01, 2=Q heads23, 3=K heads23
        qkt = [[qkt_pool.tile([128, HT], F32R, tag=f"qkt{m}_{hf}", name=f"qkt{m}_{hf}")
                for hf in range(2)] for m in range(4)]
        ao = [ao_pool.tile([128, T], F32R, tag=f"ao{i}", name=f"ao{i}") for i in range(2)]

        state = {"pending": None}

        def emit_normalize(h_, half_, avsb_):
            pr_ = 64 * (h_ % 2)
            rec = rec_pool.tile([1, HT], F32R, tag="rec", name="rec")
            nc.vector.reciprocal(rec[:], avsb_[Dh:Dh + 1, :])
            bc = sc_ps.tile([128, HT], F32, tag="sc", name="bc")
            for j in range(2):
                nc.tensor.matmul(bc[0:Dh, 512 * j:512 * (j + 1)],
                                 ones64_t[:], rec[:, 512 * j:512 * (j + 1)],
                                 start=True, stop=True)
            nc.vector.tensor_mul(ao[h_ // 2][pr_:pr_ + 64, HT * half_:HT * (half_ + 1)],
                                 avsb_[0:Dh, :], bc[0:Dh, :].bitcast(F32R))

        def attn_unit(h, half, fillers=()):
            """scores^T/exp/mask/attn@V for head h, query half `half`.
            `fillers` are independent emission closures injected one-per-strip
            to keep PE fed while the softmax pipeline ramps."""
            fillers = list(fillers)
            qrmt, krmt = (0, 1) if h < 2 else (2, 3)
            pr = 64 * (h % 2)
            q_lo = HT * half
            av = av_ps.tile([128, HT], F32, tag="av", name="av")
            n_strips = 8 if half == 0 else 16

            def emit_av(m, p_, cs_):
                seg = cs_ - q_lo   # local column within av tile
                while seg < HT:
                    seg_end = min((seg // 512 + 1) * 512, HT)
                    jb = 2 * half + seg // 512
                    nc.tensor.matmul(
                        av[0:Dh + 1, seg:seg_end],
                        vext[m][:, h, :],
                        p_[:, seg - (cs_ - q_lo):seg_end - (cs_ - q_lo)],
                        start=(m == 0), stop=(m == 4 * jb + 3))
                    seg = seg_end

            prev = None
            for m in range(n_strips):
                cs = max(q_lo, 128 * m)
                W = q_lo + HT - cs
                kr_t = qkt[krmt][m // 8]
                kc = 128 * m - HT * (m // 8)
                sc = sc_ps.tile([128, HT], F32, tag="sc", name="sc")
                j = 0
                while 512 * j < W:
                    n = min(512, W - 512 * j)
                    qc = cs - q_lo + 512 * j
                    nc.tensor.matmul(
                        sc[:, 512 * j:512 * j + n],
                        kr_t[pr:pr + 64, kc:kc + 128],
                        qkt[qrmt][half][pr:pr + 64, qc:qc + n],
                        start=True, stop=True)
                    j += 1
                p = p_pool.tile([128, HT], F32R, tag="p", name="p")
                nc.scalar.activation(p[:, 0:W], sc[:, 0:W], AF.Exp, scale=0.125)
                if cs == 128 * m:
                    nc.vector.tensor_mul(p[:, 0:128], p[:, 0:128], mask_t[:])
                if m == 0 and state["pending"] is not None:
                    emit_normalize(*state["pending"])
                    state["pending"] = None
                if prev is not None:
                    emit_av(*prev)
                prev = (m, p, cs)
                if m >= 1 and fillers:
                    fillers.pop(0)()
            emit_av(*prev)
            for f in fillers:
                f()
            avsb = avsb_pool.tile([Dh + 1, HT], F32R, tag="avsb", name="avsb")
            nc.vector.tensor_copy(avsb[0:Dh + 1, :], av[0:Dh + 1, :])
            state["pending"] = (h, half, avsb)

        with tc.tile_pool(name="xtp", bufs=1) as xt_pool, \
             tc.tile_pool(name="wqkp", bufs=1) as wqk_pool, \
             tc.tile_pool(name="ropetab", bufs=1) as rtab_pool, \
             tc.tile_pool(name="ropetmp", bufs=2) as rtmp_pool, \
             tc.tile_pool(name="projps", bufs=2, space="PSUM") as proj_ps:

            wqkv_t = [wqk_pool.tile([128, 512 + HL * Dh], F32R, tag=f"wqkv{k}", name=f"wqkv{k}")
                      for k in range(CK)]
            wqk_t = [w[:, 0:512] for w in wqkv_t]
            wv_t = [w[:, 512:512 + HL * Dh] for w in wqkv_t]

            xt_t = [xt_pool.tile([128, T], F32R, tag=f"xt{k}", name=f"xt{k}")
                    for k in range(CK)]

            def xt_dma(hf):
                for k in range(CK):
                    nc.sync.dma_start(xt_t[k][:, HT * hf:HT * (hf + 1)],
                                      xt[128 * k:128 * (k + 1), HT * hf:HT * (hf + 1)])

            # DMA emission order tracks the attention-A critical path
            cos_t = rtab_pool.tile([128, T], F32R, tag="cos")
            sin_t = rtab_pool.tile([128, T], F32R, tag="sin")
            for k in range(CK):
                nc.sync.dma_start(wqkv_t[k][:], wqkv[128 * k:128 * (k + 1), :])
                nc.sync.dma_start(xt_t[k][:, 0:HT], xt[128 * k:128 * (k + 1), 0:HT])
            nc.sync.dma_start(cos_t[:], cosT[:])
            nc.sync.dma_start(sin_t[:], sinT[:])
            nc.sync.dma_start(mask_t[:], maskT[:])
            nc.sync.dma_start(rotT_t[:], rotT[:])
            nc.sync.dma_start(ones64_t[:], ones64[:])

            rope_pending = []

            def emit_rope(m, n):
                """rotate-half via a PE permutation matmul, then the cos/sin
                elementwise combine. Emitted one projection group late so the
                PSUM->SBUF copy has drained."""
                dst = qkt[m][n // 2]
                src = dst[:, 512 * (n % 2):512 * (n % 2 + 1)]
                rps = sc_ps.tile([128, 512], F32, tag="sc", name="rps")
                nc.tensor.matmul(rps[:], rotT_t[:], src, start=True, stop=True)
                rot = rtmp_pool.tile([128, 512], F32R, tag="rot", name="rot")
                nc.vector.tensor_mul(rot[:], rps[:].bitcast(F32R),
                                     sin_t[:, 512 * n:512 * (n + 1)])
                nc.gpsimd.tensor_mul(src, src, cos_t[:, 512 * n:512 * (n + 1)])
                nc.gpsimd.tensor_add(src, src, rot[:])

            def flush_rope():
                while rope_pending:
                    emit_rope(*rope_pending.pop(0))

            def proj_group(m, n):
                pp = proj_ps.tile([128, 512], F32, tag="pp", name="pp")
                for k in range(CK):
                    nc.tensor.matmul(pp[:], wqk_t[k][:, 128 * m:128 * (m + 1)],
                                     xt_t[k][:, 512 * n:512 * (n + 1)],
                                     start=(k == 0), stop=(k == CK - 1))
                dst = qkt[m][n // 2]
                nc.vector.tensor_copy(dst[:, 512 * (n % 2):512 * (n % 2 + 1)], pp[:])
                pending = rope_pending[:]
                rope_pending.clear()
                rope_pending.append((m, n))
                for pmn in pending:
                    emit_rope(*pmn)

            def vproj_tile(t_):
                flush_rope()
                vp = proj_ps.tile([128, HL * Dh], F32, tag="pp", name="vp")
                for k in range(CK):
                    nc.tensor.matmul(vp[:], xt_t[k][:, 128 * t_:128 * (t_ + 1)], wv_t[k][:],
                                     start=(k == 0), stop=(k == CK - 1))
                nc.vector.tensor_copy(
                    vext[t_][:, :, 0:Dh],
                    vp[:].rearrange("p (h d) -> p h d", h=HL))
                nc.sync.dma_start(
                    vext[t_][:, :, Dh:Dh + 1],
                    ones4[:, HL * t_:HL * (t_ + 1)].rearrange("p (h x) -> p h x", x=1))

            # heads01 projections + V for the first query half, then attention-A
            # units with the remaining projection work injected between strips
            # (PE executes in emission order, so attention must be emitted as
            # soon as its dependencies are, with later work woven in as filler)
            def pg(m, n):
                return lambda: proj_group(m, n)

            def vt(t_):
                return lambda: vproj_tile(t_)

            proj_group(0, 0)
            proj_group(1, 0)
            proj_group(0, 1)
            proj_group(1, 1)
            for t_ in range(0, 4):
                vproj_tile(t_)
            attn_unit(0, 0, [vt(4), vt(5), vt(6), vt(7), pg(2, 0), pg(2, 1)])
            xt_dma(1)
            attn_unit(1, 0, [pg(3, 0), pg(3, 1), pg(0, 2), pg(1, 2), vt(8), vt(9)])
            attn_unit(2, 0, [vt(10), vt(11), pg(0, 3), pg(1, 3), vt(12), vt(13)])
            attn_unit(3, 0, [vt(14), vt(15), pg(2, 2), pg(2, 3), pg(3, 2), pg(3, 3)])
            flush_rope()

        with tc.tile_pool(name="wop", bufs=1) as wo_pool, \
             tc.tile_pool(name="outsb", bufs=3) as out_pool, \
             tc.tile_pool(name="opps", bufs=2, space="PSUM") as op_ps:
            wo_t = [wo_pool.tile([128, C], F32R, tag=f"wo{i}", name=f"wo{i}")
                    for i in range(2)]
            for i in range(2):
                nc.sync.dma_start(wo_t[i][:], wo[128 * i:128 * (i + 1), :])

            def outproj_tile(t_, engine, alt=False):
                osb = out_pool.tile([128, C], F32, tag="osb", name="osb")
                for n in range(2):
                    if alt:
                        op = sc_ps.tile([128, 512], F32, tag="sc", name="op")
                    else:
                        op = op_ps.tile([128, 512], F32, tag="op", name="op")
                    nc.tensor.matmul(op[:],
                                     ao[0][:, 128 * t_:128 * (t_ + 1)],
                                     wo_t[0][:, 512 * n:512 * (n + 1)],
                                     start=True, stop=False)
                    nc.tensor.matmul(op[:],
                                     ao[1][:, 128 * t_:128 * (t_ + 1)],
                                     wo_t[1][:, 512 * n:512 * (n + 1)],
                                     start=False, stop=True)
                    if engine == "v":
                        nc.vector.tensor_copy(osb[:, 512 * n:512 * (n + 1)], op[:])
                    else:
                        nc.scalar.copy(osb[:, 512 * n:512 * (n + 1)], op[:])
                nc.sync.dma_start(out[128 * t_:128 * (t_ + 1), :], osb[:])

            def ot(t_, eng):
                return lambda: outproj_tile(t_, eng)

            attn_unit(0, 1)
            attn_unit(1, 1, [ot(0, "v"), ot(1, "v"), ot(2, "v")])
            attn_unit(2, 1, [ot(3, "v"), ot(4, "v"), ot(5, "v")])
            attn_unit(3, 1, [ot(6, "v"), ot(7, "v")])
            emit_normalize(*state["pending"])
            state["pending"] = None
            for t_ in range(8, TTL):
                outproj_tile(t_, "s" if t_ % 2 == 0 else "v", alt=(t_ % 2 == 1))

    nc.finalize()
    return nc


_NC = None


def _get_nc():
    global _NC
    if _NC is None:
        _NC = build_nc()
    return _NC


def _host_tables():
    inv_freq = 1.0 / (10000.0 ** (np.arange(0, Dh, 2, dtype=np.float32) / Dh))  # [32]
    t = np.arange(T, dtype=np.float32)
    freqs = t[:, None] * inv_freq[None, :]                  # [T, 32]
    emb = np.concatenate([freqs, freqs], axis=-1)           # [T, 64]
    cos = np.cos(emb).T.astype(np.float32)                  # [64, T]
    sin = np.sin(emb).T.astype(np.float32)                  # [64, T]
    sin_signed = sin.copy()
    sin_signed[0:32, :] *= -1.0                             # rotate_half sign fold
    cosT = np.concatenate([cos, cos], axis=0)               # [128, T] two head-halves
    sinT = np.concatenate([sin_signed, sin_signed], axis=0)
    maskT = np.triu(np.ones((128, 128), np.float32))        # keep where k <= q
    sigma = np.empty(64, np.int64)
    sigma[0:32] = 2 * np.arange(32) + 1
    sigma[32:64] = 2 * np.arange(32)
    R = np.zeros((128, 128), np.float32)
    for hh in range(2):
        for d in range(64):
            R[64 * hh + d, 64 * hh + sigma[d]] = 1.0
    rotT = np.ascontiguousarray(R.T)
    return cosT, sinT, maskT, rotT


def kernel(x, w_qkv, w_out):
    x = np.asarray(x, dtype=np.float32)
    w_qkv = np.asarray(w_qkv, dtype=np.float32)
    w_out = np.asarray(w_out, dtype=np.float32)
    nc = _get_nc()
    cosT, sinT, maskT, rotT = _host_tables()
    ones64 = np.ones((1, Dh), np.float32)
    ones4 = np.ones((128, TTL * HL), np.float32)

    in_maps = []
    for core in range(N_CORES):
        b = core // 4
        g = core % 4
        heads = [4 * g + l for l in range(HL)]
        qcols = [w_qkv[:, 64 * h:64 * (h + 1)] for h in heads]
        kcols = [w_qkv[:, C + 64 * h:C + 64 * (h + 1)] for h in heads]
        vcols = [w_qkv[:, 2 * C + 64 * h:2 * C + 64 * (h + 1)] for h in heads]
        # m-tiles: Q01 | K01 | Q23 | K23
        wqkv_loc = np.concatenate(
            [qcols[0], qcols[1], kcols[0], kcols[1], qcols[2], qcols[3], kcols[2], kcols[3]]
            + vcols, axis=1).copy()                          # [C, 768]
        wo_loc = np.concatenate([w_out[64 * h:64 * (h + 1), :] for h in heads], axis=0).copy()
        in_maps.append({
            "xt": np.ascontiguousarray(x[b].T),              # [C, T]
            "wqkv": wqkv_loc,
            "wo": wo_loc,
            "cosT": cosT, "sinT": sinT, "maskT": maskT, "rotT": rotT,
            "ones64": ones64, "ones4": ones4,
        })

    res = run_bass_kernel_spmd(nc, in_maps, core_ids=list(range(N_CORES)))
    out_arr = np.zeros((B, T, C), np.float32)
    for core in range(N_CORES):
        out_arr[core // 4] += res.results[core]["out"]
    return out_arr


# revision 22
# speedup vs baseline: 1.0027x; 1.0006x over previous
"""Multi-head self-attention (RoPE, causal) Trainium2 kernel, 8-way sharded.

Sharding: data-parallel over batch (B=2) x tensor-parallel over head groups
(16 heads -> 4 groups of 4). Core c handles batch c//4, heads 4*(c%4)..+4.
Each core computes q/k/v projections for its heads, RoPE, causal-softmax
attention, and a Megatron-style row-parallel partial of the output
projection; the host sums the 4 partials per batch.

On-device layout trick: scores are computed transposed (scores^T[k, q]) so
that softmax-denominator accumulation and the attn@V contraction both run
as natural matmuls with zero on-chip transposes. x is fed pre-transposed
[C, T] from the host; rotate_half is an SBUF->SBUF partition-gather DMA
with the sign folded into the sin table. Attention is processed in two
query halves so the first half starts while input DMA/projections for the
second half are still in flight, and the first half of the output
projection overlaps the second attention half.
"""
import sys
for _p in ("/opt/trn_rl_repo",):
    if _p not in sys.path:
        sys.path.insert(0, _p)

import numpy as np
from contextlib import ExitStack

import concourse.bacc as bacc
import concourse.mybir as mybir
import concourse.tile as tile
from concourse.bass_utils import run_bass_kernel_spmd

F32 = mybir.dt.float32
F32R = mybir.dt.float32r
AF = mybir.ActivationFunctionType

B, T, C = 2, 2048, 1024
H, Dh = 16, 64
HL = 4                      # heads per core
CK = C // 128               # 8 contraction k-tiles for projections
TTL = T // 128              # 16 T-tiles / kv k-tiles
HT = T // 2                 # 1024, the attention q-half width
N_CORES = 8


def build_nc():
    nc = bacc.Bacc("TRN2", target_bir_lowering=False, debug=False, num_devices=N_CORES)

    xt = nc.declare_dram_parameter("xt", [C, T], F32R, isOutput=False)
    wqkv = nc.declare_dram_parameter("wqkv", [C, 4 * 128 + HL * Dh], F32R, isOutput=False)
    wo = nc.declare_dram_parameter("wo", [HL * Dh, C], F32R, isOutput=False)
    cosT = nc.declare_dram_parameter("cosT", [128, T], F32R, isOutput=False)
    sinT = nc.declare_dram_parameter("sinT", [128, T], F32R, isOutput=False)
    maskT = nc.declare_dram_parameter("maskT", [128, 128], F32R, isOutput=False)
    ones64 = nc.declare_dram_parameter("ones64", [1, Dh], F32R, isOutput=False)
    ones4 = nc.declare_dram_parameter("ones4", [128, TTL * HL], F32R, isOutput=False)
    rotT = nc.declare_dram_parameter("rotT", [128, 128], F32R, isOutput=False)
    out = nc.declare_dram_parameter("out", [T, C], F32, isOutput=True)

    with nc.allow_low_precision("fp32r matmul pipeline"), \
         tile.TileContext(nc) as tc, ExitStack() as octx:
        consts = octx.enter_context(tc.tile_pool(name="consts", bufs=1))
        v_pool = octx.enter_context(tc.tile_pool(name="v", bufs=1))
        qkt_pool = octx.enter_context(tc.tile_pool(name="qkt", bufs=1))
        ao_pool = octx.enter_context(tc.tile_pool(name="ao", bufs=1))
        p_pool = octx.enter_context(tc.tile_pool(name="pb", bufs=3))
        avsb_pool = octx.enter_context(tc.tile_pool(name="avsbp", bufs=2))
        rec_pool = octx.enter_context(tc.tile_pool(name="recp", bufs=2))
        sc_ps = octx.enter_context(tc.tile_pool(name="scps", bufs=2, space="PSUM"))
        av_ps = octx.enter_context(tc.tile_pool(name="avps", bufs=1, space="PSUM"))

        mask_t = consts.tile([128, 128], F32R, tag="mask")
        rotT_t = consts.tile([128, 128], F32R, tag="rotT")
        ones64_t = consts.tile([1, Dh], F32R, tag="ones64")

        vext_t = v_pool.tile([128, TTL, HL, Dh + 1], F32R, tag="vext", name="vext")
        vext = [vext_t[:, t_] for t_ in range(TTL)]
        # qkt[mt][half]: mt 0=Q heads01, 1=K heads01, 2=Q heads23, 3=K heads23
        qkt = [[qkt_pool.tile([128, HT], F32R, tag=f"qkt{m}_{hf}", name=f"qkt{m}_{hf}")
                for hf in range(2)] for m in range(4)]
        ao = [ao_pool.tile([128, T], F32R, tag=f"ao{i}", name=f"ao{i}") for i in range(2)]

        state = {"pending": None}

        def emit_normalize(h_, half_, avsb_):
            pr_ = 64 * (h_ % 2)
            rec = rec_pool.tile([1, HT], F32R, tag="rec", name="rec")
            nc.vector.reciprocal(rec[:], avsb_[Dh:Dh + 1, :])
            bc = sc_ps.tile([128, HT], F32, tag="sc", name="bc")
            for j in range(2):
                nc.tensor.matmul(bc[0:Dh, 512 * j:512 * (j + 1)],
                                 ones64_t[:], rec[:, 512 * j:512 * (j + 1)],
                                 start=True, stop=True)
            nc.vector.tensor_mul(ao[h_ // 2][pr_:pr_ + 64, HT * half_:HT * (half_ + 1)],
                                 avsb_[0:Dh, :], bc[0:Dh, :].bitcast(F32R))

        def attn_unit(h, half, fillers=()):
            """scores^T/exp/mask/attn@V for head h, query half `half`.
            `fillers` are independent emission closures injected one-per-strip
            to keep PE fed while the softmax pipeline ramps."""
            fillers = list(fillers)
            qrmt, krmt = (0, 1) if h < 2 else (2, 3)
            pr = 64 * (h % 2)
            q_lo = HT * half
            av = av_ps.tile([128, HT], F32, tag="av", name="av")
            n_strips = 8 if half == 0 else 16

            def emit_av(m, p_, cs_):
                seg = cs_ - q_lo   # local column within av tile
                while seg < HT:
                    seg_end = min((seg // 512 + 1) * 512, HT)
                    jb = 2 * half + seg // 512
                    nc.tensor.matmul(
                        av[0:Dh + 1, seg:seg_end],
                        vext[m][:, h, :],
                        p_[:, seg - (cs_ - q_lo):seg_end - (cs_ - q_lo)],
                        start=(m == 0), stop=(m == 4 * jb + 3))
                    seg = seg_end

            prev = None
            for m in range(n_strips):
                cs = max(q_lo, 128 * m)
                W = q_lo + HT - cs
                kr_t = qkt[krmt][m // 8]
                kc = 128 * m - HT * (m // 8)
                sc = sc_ps.tile([128, HT], F32, tag="sc", name="sc")
                j = 0
                while 512 * j < W:
                    n = min(512, W - 512 * j)
                    qc = cs - q_lo + 512 * j
                    nc.tensor.matmul(
                        sc[:, 512 * j:512 * j + n],
                        kr_t[pr:pr + 64, kc:kc + 128],
                        qkt[qrmt][half][pr:pr + 64, qc:qc + n],
                        start=True, stop=True)
                    j += 1
                p = p_pool.tile([128, HT], F32R, tag="p", name="p")
                nc.scalar.activation(p[:, 0:W], sc[:, 0:W], AF.Exp, scale=0.125)
                if cs == 128 * m:
                    nc.vector.tensor_mul(p[:, 0:128], p[:, 0:128], mask_t[:])
                if m == 0 and state["pending"] is not None:
                    emit_normalize(*state["pending"])
                    state["pending"] = None
                if prev is not None:
                    emit_av(*prev)
                prev = (m, p, cs)
                if m >= 1 and fillers:
                    fillers.pop(0)()
            emit_av(*prev)
            avsb = avsb_pool.tile([Dh + 1, HT], F32R, tag="avsb", name="avsb")
            nc.vector.tensor_copy(avsb[0:Dh + 1, :], av[0:Dh + 1, :])
            state["pending"] = (h, half, avsb)
            for f in fillers:
                f()

        with tc.tile_pool(name="xtp", bufs=1) as xt_pool, \
             tc.tile_pool(name="wqkp", bufs=1) as wqk_pool, \
             tc.tile_pool(name="ropetab", bufs=1) as rtab_pool, \
             tc.tile_pool(name="ropetmp", bufs=2) as rtmp_pool, \
             tc.tile_pool(name="projps", bufs=2, space="PSUM") as proj_ps:

            wqkv_t = [wqk_pool.tile([128, 512 + HL * Dh], F32R, tag=f"wqkv{k}", name=f"wqkv{k}")
                      for k in range(CK)]
            wqk_t = [w[:, 0:512] for w in wqkv_t]
            wv_t = [w[:, 512:512 + HL * Dh] for w in wqkv_t]

            xt_t = [xt_pool.tile([128, T], F32R, tag=f"xt{k}", name=f"xt{k}")
                    for k in range(CK)]

            def xt_dma(hf):
                for k in range(CK):
                    nc.sync.dma_start(xt_t[k][:, HT * hf:HT * (hf + 1)],
                                      xt[128 * k:128 * (k + 1), HT * hf:HT * (hf + 1)])

            # DMA emission order tracks the attention-A critical path
            cos_t = rtab_pool.tile([128, T], F32R, tag="cos")
            sin_t = rtab_pool.tile([128, T], F32R, tag="sin")
            for k in range(CK):
                nc.sync.dma_start(wqkv_t[k][:], wqkv[128 * k:128 * (k + 1), :])
                nc.sync.dma_start(xt_t[k][:, 0:HT], xt[128 * k:128 * (k + 1), 0:HT])
            nc.sync.dma_start(cos_t[:], cosT[:])
            nc.sync.dma_start(sin_t[:], sinT[:])
            nc.sync.dma_start(mask_t[:], maskT[:])
            nc.sync.dma_start(rotT_t[:], rotT[:])
            nc.sync.dma_start(ones64_t[:], ones64[:])

            rope_pending = []

            def emit_rope(m, n):
                """rotate-half via a PE permutation matmul, then the cos/sin
                elementwise combine. Emitted one projection group late so the
                PSUM->SBUF copy has drained."""
                dst = qkt[m][n // 2]
                src = dst[:, 512 * (n % 2):512 * (n % 2 + 1)]
                rps = sc_ps.tile([128, 512], F32, tag="sc", name="rps")
                nc.tensor.matmul(rps[:], rotT_t[:], src, start=True, stop=True)
                rot = rtmp_pool.tile([128, 512], F32R, tag="rot", name="rot")
                nc.vector.tensor_mul(rot[:], rps[:].bitcast(F32R),
                                     sin_t[:, 512 * n:512 * (n + 1)])
                nc.gpsimd.tensor_mul(src, src, cos_t[:, 512 * n:512 * (n + 1)])
                nc.gpsimd.tensor_add(src, src, rot[:])

            def flush_rope():
                while rope_pending:
                    emit_rope(*rope_pending.pop(0))

            def proj_group(m, n):
                pp = proj_ps.tile([128, 512], F32, tag="pp", name="pp")
                for k in range(CK):
                    nc.tensor.matmul(pp[:], wqk_t[k][:, 128 * m:128 * (m + 1)],
                                     xt_t[k][:, 512 * n:512 * (n + 1)],
                                     start=(k == 0), stop=(k == CK - 1))
                dst = qkt[m][n // 2]
                nc.vector.tensor_copy(dst[:, 512 * (n % 2):512 * (n % 2 + 1)], pp[:])
                pending = rope_pending[:]
                rope_pending.clear()
                rope_pending.append((m, n))
                for pmn in pending:
                    emit_rope(*pmn)

            def vproj_tile(t_):
                flush_rope()
                vp = proj_ps.tile([128, HL * Dh], F32, tag="pp", name="vp")
                for k in range(CK):
                    nc.tensor.matmul(vp[:], xt_t[k][:, 128 * t_:128 * (t_ + 1)], wv_t[k][:],
                                     start=(k == 0), stop=(k == CK - 1))
                nc.vector.tensor_copy(
                    vext[t_][:, :, 0:Dh],
                    vp[:].rearrange("p (h d) -> p h d", h=HL))
                nc.sync.dma_start(
                    vext[t_][:, :, Dh:Dh + 1],
                    ones4[:, HL * t_:HL * (t_ + 1)].rearrange("p (h x) -> p h x", x=1))

            # heads01 projections + V for the first query half, then attention-A
            # units with the remaining projection work injected between strips
            # (PE executes in emission order, so attention must be emitted as
            # soon as its dependencies are, with later work woven in as filler)
            def pg(m, n):
                return lambda: proj_group(m, n)

            def vt(t_):
                return lambda: vproj_tile(t_)

            proj_group(0, 0)
            proj_group(1, 0)
            proj_group(0, 1)
            proj_group(1, 1)
            for t_ in range(0, 4):
                vproj_tile(t_)
            attn_unit(0, 0, [vt(4), vt(5), vt(6), vt(7), pg(2, 0), pg(2, 1)])
            xt_dma(1)
            attn_unit(1, 0, [pg(3, 0), pg(3, 1), pg(0, 2), pg(1, 2), vt(8), vt(9)])
            attn_unit(2, 0, [vt(10), vt(11), pg(0, 3), pg(1, 3), vt(12), vt(13)])
            attn_unit(3, 0, [vt(14), vt(15), pg(2, 2), pg(2, 3), pg(3, 2), pg(3, 3)])
            flush_rope()

        with tc.tile_pool(name="wop", bufs=1) as wo_pool, \
             tc.tile_pool(name="outsb", bufs=3) as out_pool, \
             tc.tile_pool(name="opps", bufs=2, space="PSUM") as op_ps:
            wo_t = [wo_pool.tile([128, C], F32R, tag=f"wo{i}", name=f"wo{i}")
                    for i in range(2)]
            for i in range(2):
                nc.sync.dma_start(wo_t[i][:], wo[128 * i:128 * (i + 1), :])

            def outproj_tile(t_, engine, alt=False):
                osb = out_pool.tile([128, C], F32, tag="osb", name="osb")
                for n in range(2):
                    if alt:
                        op = sc_ps.tile([128, 512], F32, tag="sc", name="op")
                    else:
                        op = op_ps.tile([128, 512], F32, tag="op", name="op")
                    nc.tensor.matmul(op[:],
                                     ao[0][:, 128 * t_:128 * (t_ + 1)],
                                     wo_t[0][:, 512 * n:512 * (n + 1)],
                                     start=True, stop=False)
                    nc.tensor.matmul(op[:],
                                     ao[1][:, 128 * t_:128 * (t_ + 1)],
                                     wo_t[1][:, 512 * n:512 * (n + 1)],
                                     start=False, stop=True)
                    if engine == "v":
                        nc.vector.tensor_copy(osb[:, 512 * n:512 * (n + 1)], op[:])
                    else:
                        nc.scalar.copy(osb[:, 512 * n:512 * (n + 1)], op[:])
                nc.sync.dma_start(out[128 * t_:128 * (t_ + 1), :], osb[:])

            def ot(t_, eng):
                return lambda: outproj_tile(t_, eng)

            attn_unit(0, 1)
            attn_unit(1, 1, [ot(0, "v"), ot(1, "v"), ot(2, "v")])
            attn_unit(2, 1, [ot(3, "v"), ot(4, "v"), ot(5, "v")])
            attn_unit(3, 1, [ot(6, "v"), ot(7, "v")])
            emit_normalize(*state["pending"])
            state["pending"] = None
            for t_ in range(8, TTL):
                outproj_tile(t_, "s", alt=(t_ % 2 == 1))

    nc.finalize()
    return nc


_NC = None


def _get_nc():
    global _NC
    if _NC is None:
        _NC = build_nc()
    return _NC


def _host_tables():
    inv_freq = 1.0 / (10000.0 ** (np.arange(0, Dh, 2, dtype=np.float32) / Dh))  # [32]
    t = np.arange(T, dtype=np.float32)
    freqs = t[:, None] * inv_freq[None, :]                  # [T, 32]
    emb = np.concatenate([freqs, freqs], axis=-1)           # [T, 64]
    cos = np.cos(emb).T.astype(np.float32)                  # [64, T]
    sin = np.sin(emb).T.astype(np.float32)                  # [64, T]
    sin_signed = sin.copy()
    sin_signed[0:32, :] *= -1.0                             # rotate_half sign fold
    cosT = np.concatenate([cos, cos], axis=0)               # [128, T] two head-halves
    sinT = np.concatenate([sin_signed, sin_signed], axis=0)
    maskT = np.triu(np.ones((128, 128), np.float32))        # keep where k <= q
    sigma = np.empty(64, np.int64)
    sigma[0:32] = 2 * np.arange(32) + 1
    sigma[32:64] = 2 * np.arange(32)
    R = np.zeros((128, 128), np.float32)
    for hh in range(2):
        for d in range(64):
            R[64 * hh + d, 64 * hh + sigma[d]] = 1.0
    rotT = np.ascontiguousarray(R.T)
    return cosT, sinT, maskT, rotT


def kernel(x, w_qkv, w_out):
    x = np.asarray(x, dtype=np.float32)
    w_qkv = np.asarray(w_qkv, dtype=np.float32)
    w_out = np.asarray(w_out, dtype=np.float32)
    nc = _get_nc()
    cosT, sinT, maskT, rotT = _host_tables()
    ones64 = np.ones((1, Dh), np.float32)
    ones4 = np.ones((128, TTL * HL), np.float32)

    in_maps = []
    for core in range(N_CORES):
        b = core // 4
        g = core % 4
        heads = [4 * g + l for l in range(HL)]
        qcols = [w_qkv[:, 64 * h:64 * (h + 1)] for h in heads]
        kcols = [w_qkv[:, C + 64 * h:C + 64 * (h + 1)] for h in heads]
        vcols = [w_qkv[:, 2 * C + 64 * h:2 * C + 64 * (h + 1)] for h in heads]
        # m-tiles: Q01 | K01 | Q23 | K23
        wqkv_loc = np.concatenate(
            [qcols[0], qcols[1], kcols[0], kcols[1], qcols[2], qcols[3], kcols[2], kcols[3]]
            + vcols, axis=1).copy()                          # [C, 768]
        wo_loc = np.concatenate([w_out[64 * h:64 * (h + 1), :] for h in heads], axis=0).copy()
        in_maps.append({
            "xt": np.ascontiguousarray(x[b].T),              # [C, T]
            "wqkv": wqkv_loc,
            "wo": wo_loc,
            "cosT": cosT, "sinT": sinT, "maskT": maskT, "rotT": rotT,
            "ones64": ones64, "ones4": ones4,
        })

    res = run_bass_kernel_spmd(nc, in_maps, core_ids=list(range(N_CORES)))
    out_arr = np.zeros((B, T, C), np.float32)
    for core in range(N_CORES):
        out_arr[core // 4] += res.results[core]["out"]
    return out_arr
